# revision 47
# baseline (speedup 1.0000x reference)
"""
Multi-head attention (B=2, S=2048, D=1024, H=16, causal) on 8 Trainium2
NeuronCores via Bass/Tile.

Sharding: batch x head-quad. Core (b, Q) owns batch b and heads
[4Q, 4Q+4) (two groups of 2 heads), so each core reads only its batch's
q/k/v inputs (12 MB bf16 vs 24 MB for pure head sharding) and writes a
[2048, 1024] bf16 partial output. The host sums the 4 quad-partials per
batch and adds bo (the "all-reduce after w_o" done host-side since the
kernel contract is full-in / full-out).

All matmuls run in bf16 (2x the fp32r rate, FWL weight loads):
  Q^T/K^T/V^T   [128 feats (2 heads x 64), g, 2048 tokens]
  scores        S^T tiles [128 keys, q] via row-packed per-head matmuls
  softmax       exp on ACT (scale=1/8 folded in, no max-subtraction:
                |s/8| <~ 6), row sums via a ones column appended to V
                (psum row 64), reciprocal_approx_fast on DVE, partition
                broadcast via a K=2 selector matmul
  P@V           vaug [128 tokens, 65] stationary per head; diagonal
                tiles stream only the live [off:512] columns
  out proj      otn [128 feats, 512 q] merged across heads -> K=128
                matmuls against wo [128, 1024], accumulated over groups
"""

import os
import sys

for _p in ("/opt/trn_rl_repo", "/root/.axon_site/_ro/trn_rl_repo"):
    if os.path.isdir(_p) and _p not in sys.path:
        sys.path.insert(0, _p)

import numpy as np
import ml_dtypes
from contextlib import ExitStack

import concourse.bass as bass
import concourse.tile as tile
from concourse import bacc
from concourse import mybir

B, S, D, H = 2, 2048, 1024, 16
DK = D // H                # 64
NCORES = 8
NQUAD = 4                  # head quads
HPQ = H // NQUAD           # 4 heads per core
NG = 2                     # feature groups per core (2 heads each)
DG = 2 * DK                # 128 feats per group
SCALE = 1.0 / np.sqrt(DK)  # 0.125

KC = D // 128              # 8 contraction chunks for projections
NJP = S // 1024            # 2 token chunks of 1024
NQJ = S // 512             # 4 q chunks of 512
NKT = S // 128             # 16 k tiles of 128

F32 = mybir.dt.float32
F32R = mybir.dt.float32r
BF16 = mybir.dt.bfloat16


def build_kernel(mode="causal", dbg=False):
    """Identical program on all cores; per-core slices arrive as data.

    mode: "causal" (skip upper-triangular key tiles, tri-mask diagonal),
          "ones" (no masking), "general" (additive mask streamed from DRAM).
    """
    nc = bacc.Bacc()

    xq = nc.declare_dram_parameter("xq", [D, S], BF16, isOutput=False)
    xk = nc.declare_dram_parameter("xk", [D, S], BF16, isOutput=False)
    xv = nc.declare_dram_parameter("xv", [D, S], BF16, isOutput=False)
    wq = nc.declare_dram_parameter("wq", [D, NG, DG], BF16, isOutput=False)
    wk = nc.declare_dram_parameter("wk", [D, NG, DG], BF16, isOutput=False)
    wv = nc.declare_dram_parameter("wv", [D, NG, DG], BF16, isOutput=False)
    wqb = nc.declare_dram_parameter("wqb", [DG, NG], F32, isOutput=False)
    wkb = nc.declare_dram_parameter("wkb", [DG, NG], F32, isOutput=False)
    wvb = nc.declare_dram_parameter("wvb", [DG, NG], F32, isOutput=False)
    wo = nc.declare_dram_parameter("wo", [NG, DG, D], BF16, isOutput=False)
    tri = nc.declare_dram_parameter("tri", [128, 128], BF16, isOutput=False)
    idn = nc.declare_dram_parameter("idn", [128, 128], BF16, isOutput=False)
    onesm = nc.declare_dram_parameter("onesm", [128, 64], BF16, isOutput=False)
    onesr = nc.declare_dram_parameter("onesr", [1, 64], F32R, isOutput=False)
    madd = None
    if mode == "general":
        madd = nc.declare_dram_parameter("madd", [S, S], F32, isOutput=False)
    out = nc.declare_dram_parameter("out", [S, D], BF16, isOutput=True)
    dbg_t = {}
    if dbg:
        for dn, shape, dt in (
                ("dq", [128, NG * S], BF16), ("dk", [128, NG * S], BF16),
                ("dv", [128, NG * S], BF16),
                ("dvaug", [128, NG * 2 * NKT * (DK + 1)], BF16),
                ("dst", [128, 1024], F32), ("dpt", [128, 1024], BF16),
                ("dot", [DK + 1, 1024], F32),
                ("drbc", [128, 512], F32),
                ("dotn", [128, NG * NQJ * 512], BF16)):
            dbg_t[dn] = nc.declare_dram_parameter(dn, shape, dt, isOutput=True)

    def n_keytiles(qj):
        return 4 * qj + 4 if mode == "causal" else NKT

    with tile.TileContext(nc) as tc, ExitStack() as ctx:
        persist = ctx.enter_context(tc.tile_pool(name="persist", bufs=1))
        ptp = ctx.enter_context(tc.tile_pool(name="ptp", bufs=3))
        rcp = ctx.enter_context(tc.tile_pool(name="rcp", bufs=2))
        out_p = ctx.enter_context(tc.tile_pool(name="outp", bufs=4))
        mk_p = None
        if mode == "general":
            mk_p = ctx.enter_context(tc.tile_pool(name="mk", bufs=4))
        # PSUM: st2 2 banks x 2 bufs + ot 1 bank x 2 + po 1 + pp 1 = 8 banks
        st2 = ctx.enter_context(
            tc.tile_pool(name="st2", bufs=2, space=bass.MemorySpace.PSUM))
        otps = ctx.enter_context(
            tc.tile_pool(name="otps", bufs=2, space=bass.MemorySpace.PSUM))
        po = ctx.enter_context(
            tc.tile_pool(name="po", bufs=1, space=bass.MemorySpace.PSUM))
        pp_p = ctx.enter_context(
            tc.tile_pool(name="ppp", bufs=1, space=bass.MemorySpace.PSUM))

        # ---------------- persistent tiles ----------------
        qt = persist.tile([128, NG, S], BF16)       # Q^T
        kt = persist.tile([128, NG, S], BF16)       # K^T
        vt = persist.tile([128, NG, S], BF16)       # V^T (consumed by transpose)
        # V augmented: [128 tokens, g, head, ktile, 65]; col 64 == 1.0
        vaug = persist.tile([128, NG, 2, NKT, DK + 1], BF16)
        # normalized attention outputs, [128 feats(2 heads), g, qj, 512 q]
        otn = persist.tile([128, NG, NQJ, 512], BF16)
        wo_sb = persist.tile([128, NG, D], BF16)
        tri_sb = persist.tile([128, 128], BF16)
        ident = persist.tile([128, 128], BF16)
        ones_sb = persist.tile([128, 64], BF16)
        onesr_sb = persist.tile([1, 64], F32R)
        xq_sb = persist.tile([128, KC, S], BF16)
        xk_sb = persist.tile([128, KC, S], BF16)
        xv_sb = persist.tile([128, KC, S], BF16)

        # DMA emission order = consumption order. Each DMA_DIRECT2D costs
        # ~0.6us of Sync-engine issue time, so the first projection's
        # operands (wk + xk token-half 0) go first; constants and token-half
        # 1 (as one large transfer per tensor) follow.
        w_sb = {}
        wb_sb = {}
        xsbs = {"q": xq_sb, "k": xk_sb, "v": xv_sb}
        xsrcs = {"q": xq, "k": xk, "v": xv}

        def load_w(name, wsrc, wbsrc):
            wt = persist.tile([128, KC, NG, DG], BF16, name=f"w{name}")
            nc.sync.dma_start(
                out=wt, in_=wsrc[:, :, :].rearrange("(c p) g n -> p c g n",
                                                    p=128))
            bt = persist.tile([DG, NG], F32, name=f"wb{name}")
            nc.sync.dma_start(out=bt, in_=wbsrc[:, :])
            w_sb[name] = wt
            wb_sb[name] = bt

        def load_x_chunks(name, nj):
            for c in range(KC):
                nc.sync.dma_start(
                    out=xsbs[name][:, c, nj * 1024:(nj + 1) * 1024],
                    in_=xsrcs[name][c * 128:(c + 1) * 128,
                                    nj * 1024:(nj + 1) * 1024])

        def load_x_half(name, nj):
            # one 2 MB transfer per (tensor, token-half): each DMA issue
            # costs ~0.8us of Sync time, so few big transfers beat chunks
            nc.sync.dma_start(
                out=xsbs[name][:, :, nj * 1024:(nj + 1) * 1024],
                in_=xsrcs[name][:, nj * 1024:(nj + 1) * 1024].rearrange(
                    "(c p) t -> p c t", p=128))

        load_w("k", wk, wkb)
        load_x_half("k", 0)
        load_w("v", wv, wvb)
        load_x_half("v", 0)
        nc.sync.dma_start(out=ident, in_=idn[:, :])
        load_w("q", wq, wqb)
        load_x_half("q", 0)
        nc.sync.dma_start(out=ones_sb, in_=onesm[:, :])
        nc.sync.dma_start(out=onesr_sb, in_=onesr[:, :])
        nc.sync.dma_start(out=tri_sb, in_=tri[:, :])
        for name in ("k", "v", "q"):
            load_x_half(name, 1)
        nc.sync.dma_start(out=wo_sb, in_=wo[:, :, :].rearrange("g p n -> p g n"))

        # ones column of vaug
        nc.vector.tensor_copy(
            vaug[:, :, :, :, DK:DK + 1], ones_sb[:, 0:NG * 2 * NKT])

        # ---------------- phase 1: QKV projections ----------------
        def proj(name, g, nj):
            # full [128, 1024] projection via the double-buffered st2 slots
            # (used only for the startup unit, before attention exists)
            xsb = xsbs[name]
            tgt = {"q": qt, "k": kt, "v": vt}[name]
            ps = st2.tile([128, 1024], F32, tag="st2", name=f"ps_{name}{g}{nj}")
            for c in range(KC):
                for u in range(2):
                    nc.tensor.matmul(
                        ps[:, u * 512:(u + 1) * 512],
                        w_sb[name][:, c, g, :],
                        xsb[:, c, nj * 1024 + u * 512: nj * 1024 + (u + 1) * 512],
                        start=(c == 0), stop=(c == KC - 1))
            nc.vector.tensor_scalar_add(
                tgt[:, g, nj * 1024:(nj + 1) * 1024], ps, wb_sb[name][:, g:g + 1])

        def proj_half(name, g, nj, u):
            # 512-token half projection in the single-bank "pp" slot, small
            # enough (~1.7us of PE) to slip between attention iterations
            xsb = xsbs[name]
            tgt = {"q": qt, "k": kt, "v": vt}[name]
            lo = nj * 1024 + u * 512
            ps = pp_p.tile([128, 512], F32, tag="pp", name=f"ph_{name}{g}{nj}{u}")
            for c in range(KC):
                nc.tensor.matmul(
                    ps, w_sb[name][:, c, g, :], xsb[:, c, lo:lo + 512],
                    start=(c == 0), stop=(c == KC - 1))
            nc.vector.tensor_scalar_add(
                tgt[:, g, lo:lo + 512], ps, wb_sb[name][:, g:g + 1])

        def transpose_v(g, i):
            # [128 feats, 128 tokens] -> [128 tokens, 2, 64 feats] in vaug
            trp = pp_p.tile([128, 128], BF16, tag="pp", name=f"trp{g}_{i}")
            nc.tensor.transpose(trp, vt[:, g, i * 128:(i + 1) * 128], ident)
            nc.vector.tensor_copy(
                vaug[:, g, :, i, 0:DK],
                trp[:, :].rearrange("p (h f) -> p h f", h=2))

        # startup: the minimum attn(0, 0) needs — K/V token-half 0, the
        # first 4 V-transposes, and the first 512 queries
        proj("k", 0, 0)
        proj("v", 0, 0)
        for i in range(4):
            transpose_v(0, i)
        proj_half("q", 0, 0, 0)

        # remaining projection work as a queue of small PE units, popped
        # between attention iterations so ACT never starves. Order matters
        # twice over: a popped unit whose x data hasn't landed blocks the
        # whole PE FIFO, so group (1,0) (token-half 0, resident early) goes
        # first; and group (0,1) is needed by attn(0,2) — Q at its first
        # scores, K/V/transposes at its ki=8.
        def ph(n_, g_, j_, u_):
            return lambda: proj_half(n_, g_, j_, u_)

        def tv(g_, i_):
            return lambda: transpose_v(g_, i_)

        units = [tv(0, i) for i in range(4, 8)]                # idx 0-3
        units += [ph("q", 0, 0, 1)]                            # idx 4
        for n in ("k", "v"):
            units += [ph(n, 1, 0, u) for u in range(2)]
        units += [tv(1, i) for i in range(8)]
        units += [ph("q", 1, 0, u) for u in range(2)]          # (1,0): 5-18
        units += [ph("q", 0, 1, u) for u in range(2)]          # idx 19-20
        for n in ("k", "v"):
            units += [ph(n, 0, 1, u) for u in range(2)]        # idx 21-24
        units += [tv(0, i) for i in range(8, 16)]              # idx 25-32
        for n in ("k", "v"):
            units += [ph(n, 1, 1, u) for u in range(2)]
        units += [tv(1, i) for i in range(8, 16)]
        units += [ph("q", 1, 1, u) for u in range(2)]          # (1,1): 33-46
        upos = [0]
        TOTAL_SLOTS = sum(4 * qj + 4 for qj in range(NQJ))
        slot_no = [0]

        def pop_units(force_until=None):
            if force_until is not None:
                while upos[0] < force_until:
                    units[upos[0]]()
                    upos[0] += 1
                return
            slot_no[0] += 1
            target = min(len(units),
                         (slot_no[0] * len(units) + TOTAL_SLOTS - 1) // TOTAL_SLOTS)
            while upos[0] < target:
                units[upos[0]]()
                upos[0] += 1

        # ---------------- phase 2: attention ----------------
        def attn(g, qj, pop=False, force=()):
            force = dict(force)
            n_k = n_keytiles(qj)
            ot = [otps.tile([DK + 1, 512], F32, tag="ot", name=f"ot{g}{qj}{h}")
                  for h in range(2)]
            for ki in range(n_k):
                if ki in force:
                    pop_units(force_until=force[ki])
                off = 128 * (ki - 4 * qj) if (mode == "causal" and ki >= 4 * qj) else 0
                st = st2.tile([128, 1024], F32, tag="st2", name=f"st{g}{qj}{ki}")
                for h in range(2):
                    nc.tensor.matmul(
                        st[:, h * 512 + off:(h + 1) * 512],
                        kt[h * DK:(h + 1) * DK, g, ki * 128:(ki + 1) * 128],
                        qt[h * DK:(h + 1) * DK, g, qj * 512 + off: (qj + 1) * 512],
                        start=True, stop=True,
                        tile_position=(h * DK, 0))
                if mode == "general":
                    mt = mk_p.tile([128, 512], F32, tag="mk", name=f"mt{g}{qj}{ki}")
                    nc.sync.dma_start(
                        out=mt,
                        in_=madd[ki * 128:(ki + 1) * 128,
                                 qj * 512:(qj + 1) * 512])
                    for h in range(2):
                        nc.vector.tensor_add(
                            st[:, h * 512:(h + 1) * 512],
                            st[:, h * 512:(h + 1) * 512], mt)
                pt = ptp.tile([128, 1024], BF16, tag="pt", name=f"pt{g}{qj}{ki}")
                if off == 0:
                    nc.scalar.activation(
                        pt, st, mybir.ActivationFunctionType.Exp, scale=SCALE)
                else:
                    # one strided activation covering both heads' live cols
                    nc.scalar.activation(
                        pt[:, :].rearrange("p (h q) -> p h q", h=2)[:, :, off:512],
                        st[:, :].rearrange("p (h q) -> p h q", h=2)[:, :, off:512],
                        mybir.ActivationFunctionType.Exp, scale=SCALE)
                if mode == "causal" and ki >= 4 * qj:
                    for h in range(2):
                        lo = h * 512 + off
                        nc.vector.tensor_mul(
                            pt[:, lo:lo + 128], pt[:, lo:lo + 128], tri_sb)
                if dbg and g == 0 and qj == 1 and ki == 2:
                    stg = out_p.tile([128, 1024], F32, tag="dbgst", name="dbgst",
                                     bufs=1)
                    nc.vector.tensor_copy(stg, st)
                    nc.sync.dma_start(out=dbg_t["dst"][:, :], in_=stg)
                    nc.sync.dma_start(out=dbg_t["dpt"][:, :], in_=pt)
                if pop:
                    pop_units()
                for h in range(2):
                    nc.tensor.matmul(
                        ot[h][:, off:512],
                        vaug[:, g, h, ki, :],
                        pt[:, h * 512 + off:(h + 1) * 512],
                        start=(ki == 0), stop=(ki == n_k - 1),
                        skip_group_check=True)
            # normalize: otn[h*64:(h+1)*64, g, qj, :] = ot[h][0:64] / sums.
            # Per head: copy the sum row (psum row 64) to sbuf, broadcast it
            # over 64 partitions via a K=1 f32r matmul (base 0, no col
            # tiling), approx-reciprocal at base 0, multiply into the merged
            # otn tile (only the TT *output* is partition-shifted, which the
            # plain ops handle; the custom recip op needs base-0 operands).
            if dbg and g == 0 and qj == 1:
                stg2 = out_p.tile([DK + 1, 1024], F32, tag="dbgot", name="dbgot",
                                  bufs=1)
                for h in range(2):
                    nc.vector.tensor_copy(stg2[:, h * 512:(h + 1) * 512], ot[h])
                nc.sync.dma_start(out=dbg_t["dot"][:, :], in_=stg2)
            for h in range(2):
                srow = rcp.tile([1, 512], F32R, tag=f"srow{h}",
                                name=f"srow{g}{qj}{h}")
                nc.vector.tensor_copy(srow, ot[h][DK:DK + 1, :])
                rbc_ps = po.tile([DK, 512], F32, tag="po",
                                 name=f"rbc{g}{qj}{h}")
                nc.tensor.matmul(rbc_ps, onesr_sb[0:1, 0:DK], srow,
                                 start=True, stop=True)
                rbc = rcp.tile([DK, 512], F32, tag=f"rbc{h}",
                               name=f"rbcs{g}{qj}{h}")
                nc.vector.reciprocal_approx_fast(out=rbc, in_=rbc_ps)
                if dbg and g == 0 and qj == 1:
                    nc.sync.dma_start(
                        out=dbg_t["drbc"][h * DK:(h + 1) * DK, :], in_=rbc)
                nc.vector.tensor_mul(
                    otn[h * DK:(h + 1) * DK, g, qj, :],
                    ot[h][0:DK, :], rbc)

        # attention on g0, with the remaining projection work drip-fed into
        # the PE stream between iterations so ACT never starves.
        attn(0, 0, pop=True)
        attn(0, 1, pop=True, force={0: 5})
        attn(0, 2, pop=True, force={0: 21, 8: 33})
        attn(0, 3, pop=True)
        pop_units(force_until=len(units))

        # ---------------- phase 3: attention g1 + output projection ----------------
        def oproj(qj):
            # alternate between the po and pp psum slots (pp is free once
            # the projection unit queue has drained) so matmuls on one tile
            # overlap the psum->sbuf copy of the other
            for qb in range(4):
                ob = out_p.tile([128, 1024], BF16, tag="ob", name=f"ob{qj}{qb}")
                for n in range(2):
                    pool_n = po if n == 0 else pp_p
                    pp = pool_n.tile([128, 512], F32,
                                     tag="po" if n == 0 else "pp",
                                     name=f"pp{qj}{qb}{n}")
                    for g in range(NG):
                        nc.tensor.matmul(
                            pp,
                            otn[:, g, qj, qb * 128:(qb + 1) * 128],
                            wo_sb[:, g, n * 512:(n + 1) * 512],
                            start=(g == 0), stop=(g == NG - 1))
                    nc.vector.tensor_copy(ob[:, n * 512:(n + 1) * 512], pp)
                nc.sync.dma_start(
                    out=out[qj * 512 + qb * 128: qj * 512 + (qb + 1) * 128, :],
                    in_=ob)

        # longest q-chunk first so the kernel tail is the shortest one
        for qj in reversed(range(NQJ)):
            attn(1, qj)
            oproj(qj)

        if dbg:
            nc.sync.dma_start(out=dbg_t["dq"][:, :], in_=qt[:, :, :])
            nc.sync.dma_start(out=dbg_t["dk"][:, :], in_=kt[:, :, :])
            nc.sync.dma_start(out=dbg_t["dv"][:, :], in_=vt[:, :, :])
            nc.sync.dma_start(out=dbg_t["dvaug"][:, :], in_=vaug[:, :, :, :, :])
            nc.sync.dma_start(out=dbg_t["dotn"][:, :], in_=otn[:, :, :, :])

    nc.compile()
    return nc


def detect_mode(mask):
    m = np.asarray(mask)[0, 0]
    if (m == np.tril(np.ones((S, S), m.dtype))).all():
        return "causal"
    if (m == 1).all():
        return "ones"
    return "general"


def make_core_inputs(query, key, value, mask, Wq, bq, Wk, bk, Wv, bv, Wo, bo,
                     mode="causal"):
    """Host-side sharding: returns list of per-core input dicts.

    Core c = b * NQUAD + Q owns batch b, heads [4Q, 4Q+4).
    """
    pdt = ml_dtypes.bfloat16
    tri = np.ascontiguousarray(np.triu(np.ones((128, 128), np.float32))).astype(pdt)
    idn = np.ascontiguousarray(np.eye(128, dtype=np.float32)).astype(pdt)
    onesm = np.ones((128, 64), pdt)

    xs = {}
    for b in range(B):
        xs[b] = {
            "xq": np.ascontiguousarray(np.asarray(query)[b].T.astype(pdt)),
            "xk": np.ascontiguousarray(np.asarray(key)[b].T.astype(pdt)),
            "xv": np.ascontiguousarray(np.asarray(value)[b].T.astype(pdt)),
        }
    madd_np = None
    if mode == "general":
        madd_np = np.ascontiguousarray(
            np.where(np.asarray(mask)[0, 0].T == 0, np.float32(-1e30),
                     np.float32(0.0)).astype(np.float32))

    in_maps = []
    for c in range(NCORES):
        b, Q = divmod(c, NQUAD)
        fsl = slice(Q * HPQ * DK, (Q + 1) * HPQ * DK)   # 256 feats of the quad
        m = dict(xs[b])
        m.update({
            "wq": np.ascontiguousarray(
                np.asarray(Wq)[fsl, :].T.astype(pdt).reshape(D, NG, DG)),
            "wk": np.ascontiguousarray(
                np.asarray(Wk)[fsl, :].T.astype(pdt).reshape(D, NG, DG)),
            "wv": np.ascontiguousarray(
                np.asarray(Wv)[fsl, :].T.astype(pdt).reshape(D, NG, DG)),
            "wqb": np.ascontiguousarray(
                np.asarray(bq)[fsl].astype(np.float32).reshape(NG, DG).T),
            "wkb": np.ascontiguousarray(
                np.asarray(bk)[fsl].astype(np.float32).reshape(NG, DG).T),
            "wvb": np.ascontiguousarray(
                np.asarray(bv)[fsl].astype(np.float32).reshape(NG, DG).T),
            "wo": np.ascontiguousarray(
                np.asarray(Wo)[:, fsl].T.astype(pdt).reshape(NG, DG, D)),
            "tri": tri,
            "idn": idn,
            "onesm": onesm,
            "onesr": np.ones((1, 64), np.float32),
        })
        if mode == "general":
            m["madd"] = madd_np
        in_maps.append(m)
    return in_maps


_NC_CACHE = {}


def kernel(query, key, value, mask, Wq, bq, Wk, bk, Wv, bv, Wo, bo,
           trace=False):
    from concourse.bass_utils import run_bass_kernel_spmd

    mode = detect_mode(mask)
    if mode not in _NC_CACHE:
        _NC_CACHE[mode] = build_kernel(mode=mode)
    nc = _NC_CACHE[mode]
    in_maps = make_core_inputs(
        query, key, value, mask, Wq, bq, Wk, bk, Wv, bv, Wo, bo, mode=mode)
    res = run_bass_kernel_spmd(nc, in_maps, core_ids=list(range(NCORES)),
                               trace=trace)
    out = np.zeros((B, S, D), np.float32)
    for c, r in enumerate(res.results):
        b = c // NQUAD
        out[b] += r["out"].astype(np.float32)
    out += np.asarray(bo).astype(np.float32)[None, None, :]
    if trace:
        kernel.last_results = res
    return out


# revision 49
# speedup vs baseline: 1.0952x; 1.0952x over previous
"""
Multi-head attention (B=2, S=2048, D=1024, H=16, causal) on 8 Trainium2
NeuronCores via Bass/Tile.

Sharding: batch x head-quad. Core (b, Q) owns batch b and heads
[4Q, 4Q+4) (two groups of 2 heads), so each core reads only its batch's
q/k/v inputs (12 MB bf16 vs 24 MB for pure head sharding) and writes a
[2048, 1024] bf16 partial output. The host sums the 4 quad-partials per
batch and adds bo (the "all-reduce after w_o" done host-side since the
kernel contract is full-in / full-out).

All matmuls run in bf16 (2x the fp32r rate, FWL weight loads):
  Q^T/K^T/V^T   [128 feats (2 heads x 64), g, 2048 tokens]
  scores        S^T tiles [128 keys, q] via row-packed per-head matmuls
  softmax       exp on ACT (scale=1/8 folded in, no max-subtraction:
                |s/8| <~ 6), row sums via a ones column appended to V
                (psum row 64), reciprocal_approx_fast on DVE, partition
                broadcast via a K=2 selector matmul
  P@V           vaug [128 tokens, 65] stationary per head; diagonal
                tiles stream only the live [off:512] columns
  out proj      otn [128 feats, 512 q] merged across heads -> K=128
                matmuls against wo [128, 1024], accumulated over groups
"""

import os
import sys

for _p in ("/opt/trn_rl_repo", "/root/.axon_site/_ro/trn_rl_repo"):
    if os.path.isdir(_p) and _p not in sys.path:
        sys.path.insert(0, _p)

import numpy as np
import ml_dtypes
from contextlib import ExitStack

import concourse.bass as bass
import concourse.tile as tile
from concourse import bacc
from concourse import mybir

B, S, D, H = 2, 2048, 1024, 16
DK = D // H                # 64
NCORES = 8
NQUAD = 4                  # head quads
HPQ = H // NQUAD           # 4 heads per core
NG = 2                     # feature groups per core (2 heads each)
DG = 2 * DK                # 128 feats per group
SCALE = 1.0 / np.sqrt(DK)  # 0.125

KC = D // 128              # 8 contraction chunks for projections
NJP = S // 1024            # 2 token chunks of 1024
NQJ = S // 512             # 4 q chunks of 512
NKT = S // 128             # 16 k tiles of 128

F32 = mybir.dt.float32
F32R = mybir.dt.float32r
BF16 = mybir.dt.bfloat16


def build_kernel(mode="causal", dbg=False):
    """Identical program on all cores; per-core slices arrive as data.

    mode: "causal" (skip upper-triangular key tiles, tri-mask diagonal),
          "ones" (no masking), "general" (additive mask streamed from DRAM).
    """
    nc = bacc.Bacc()

    xq = nc.declare_dram_parameter("xq", [D, S], BF16, isOutput=False)
    xk = nc.declare_dram_parameter("xk", [D, S], BF16, isOutput=False)
    xv = nc.declare_dram_parameter("xv", [D, S], BF16, isOutput=False)
    wq = nc.declare_dram_parameter("wq", [D, NG, DG], BF16, isOutput=False)
    wk = nc.declare_dram_parameter("wk", [D, NG, DG], BF16, isOutput=False)
    wv = nc.declare_dram_parameter("wv", [D, NG, DG], BF16, isOutput=False)
    wqb = nc.declare_dram_parameter("wqb", [DG, NG], F32, isOutput=False)
    wkb = nc.declare_dram_parameter("wkb", [DG, NG], F32, isOutput=False)
    wvb = nc.declare_dram_parameter("wvb", [DG, NG], F32, isOutput=False)
    wo = nc.declare_dram_parameter("wo", [NG, DG, D], BF16, isOutput=False)
    tri = nc.declare_dram_parameter("tri", [128, 128], BF16, isOutput=False)
    idn = nc.declare_dram_parameter("idn", [128, 128], BF16, isOutput=False)
    onesm = nc.declare_dram_parameter("onesm", [128, 64], BF16, isOutput=False)
    onesr = nc.declare_dram_parameter("onesr", [1, 64], F32R, isOutput=False)
    madd = None
    if mode == "general":
        madd = nc.declare_dram_parameter("madd", [S, S], F32, isOutput=False)
    out = nc.declare_dram_parameter("out", [S, D], BF16, isOutput=True)
    dbg_t = {}
    if dbg:
        for dn, shape, dt in (
                ("dq", [128, NG * S], BF16), ("dk", [128, NG * S], BF16),
                ("dv", [128, NG * S], BF16),
                ("dvaug", [128, NG * 2 * NKT * (DK + 1)], BF16),
                ("dst", [128, 1024], F32), ("dpt", [128, 1024], BF16),
                ("dot", [DK + 1, 1024], F32),
                ("drbc", [128, 512], F32),
                ("dotn", [128, NG * NQJ * 512], BF16)):
            dbg_t[dn] = nc.declare_dram_parameter(dn, shape, dt, isOutput=True)

    def n_keytiles(qj):
        return 4 * qj + 4 if mode == "causal" else NKT

    with tile.TileContext(nc) as tc, ExitStack() as ctx:
        persist = ctx.enter_context(tc.tile_pool(name="persist", bufs=1))
        ptp = ctx.enter_context(tc.tile_pool(name="ptp", bufs=3))
        rcp = ctx.enter_context(tc.tile_pool(name="rcp", bufs=2))
        out_p = ctx.enter_context(tc.tile_pool(name="outp", bufs=4))
        mk_p = None
        if mode == "general":
            mk_p = ctx.enter_context(tc.tile_pool(name="mk", bufs=4))
        # PSUM: st2 2 banks x 2 bufs + ot 1 bank x 2 + po 1 + pp 1 = 8 banks
        st2 = ctx.enter_context(
            tc.tile_pool(name="st2", bufs=2, space=bass.MemorySpace.PSUM))
        otps = ctx.enter_context(
            tc.tile_pool(name="otps", bufs=2, space=bass.MemorySpace.PSUM))
        po = ctx.enter_context(
            tc.tile_pool(name="po", bufs=1, space=bass.MemorySpace.PSUM))
        pp_p = ctx.enter_context(
            tc.tile_pool(name="ppp", bufs=1, space=bass.MemorySpace.PSUM))

        # ---------------- persistent tiles ----------------
        qt = persist.tile([128, NG, S], BF16)       # Q^T
        kt = persist.tile([128, NG, S], BF16)       # K^T
        vt = persist.tile([128, NG, S], BF16)       # V^T (consumed by transpose)
        # V augmented: [128 tokens, g, head, ktile, 65]; col 64 == 1.0
        vaug = persist.tile([128, NG, 2, NKT, DK + 1], BF16)
        # normalized attention outputs, [128 feats(2 heads), g, qj, 512 q]
        otn = persist.tile([128, NG, NQJ, 512], BF16)
        wo_sb = persist.tile([128, NG, D], BF16)
        tri_sb = persist.tile([128, 128], BF16)
        ident = persist.tile([128, 128], BF16)
        ones_sb = persist.tile([128, 64], BF16)
        onesr_sb = persist.tile([1, 64], F32R)
        xq_sb = persist.tile([128, KC, S], BF16)
        xk_sb = persist.tile([128, KC, S], BF16)
        xv_sb = persist.tile([128, KC, S], BF16)

        # DMA emission order = consumption order. Each DMA_DIRECT2D costs
        # ~0.6us of Sync-engine issue time, so the first projection's
        # operands (wk + xk token-half 0) go first; constants and token-half
        # 1 (as one large transfer per tensor) follow.
        w_sb = {}
        wb_sb = {}
        xsbs = {"q": xq_sb, "k": xk_sb, "v": xv_sb}
        xsrcs = {"q": xq, "k": xk, "v": xv}

        def load_w(name, wsrc, wbsrc):
            wt = persist.tile([128, KC, NG, DG], BF16, name=f"w{name}")
            nc.sync.dma_start(
                out=wt, in_=wsrc[:, :, :].rearrange("(c p) g n -> p c g n",
                                                    p=128))
            bt = persist.tile([DG, NG], F32, name=f"wb{name}")
            nc.sync.dma_start(out=bt, in_=wbsrc[:, :])
            w_sb[name] = wt
            wb_sb[name] = bt

        def load_x_chunks(name, nj):
            # alternate the HWDGE issuing engine (Sync / Scalar) — each
            # issue costs ~0.8us of engine time and ACT is idle at startup,
            # so splitting halves the issue-serialization latency
            for c in range(KC):
                eng = nc.sync if c % 2 == 0 else nc.scalar
                eng.dma_start(
                    out=xsbs[name][:, c, nj * 1024:(nj + 1) * 1024],
                    in_=xsrcs[name][c * 128:(c + 1) * 128,
                                    nj * 1024:(nj + 1) * 1024])

        load_w("k", wk, wkb)
        load_x_chunks("k", 0)
        load_w("v", wv, wvb)
        load_x_chunks("v", 0)
        nc.sync.dma_start(out=ident, in_=idn[:, :])
        load_w("q", wq, wqb)
        load_x_chunks("q", 0)
        nc.sync.dma_start(out=ones_sb, in_=onesm[:, :])
        nc.sync.dma_start(out=onesr_sb, in_=onesr[:, :])
        nc.sync.dma_start(out=tri_sb, in_=tri[:, :])
        # token-half 1: one 2 MB transfer per tensor
        for name in ("k", "v", "q"):
            nc.sync.dma_start(
                out=xsbs[name][:, :, 1024:2048],
                in_=xsrcs[name][:, 1024:2048].rearrange("(c p) t -> p c t",
                                                        p=128))
        nc.sync.dma_start(out=wo_sb, in_=wo[:, :, :].rearrange("g p n -> p g n"))

        # ones column of vaug
        nc.vector.tensor_copy(
            vaug[:, :, :, :, DK:DK + 1], ones_sb[:, 0:NG * 2 * NKT])

        # ---------------- phase 1: QKV projections ----------------
        def proj(name, g, nj):
            # full [128, 1024] projection via the double-buffered st2 slots
            # (used only for the startup unit, before attention exists)
            xsb = xsbs[name]
            tgt = {"q": qt, "k": kt, "v": vt}[name]
            ps = st2.tile([128, 1024], F32, tag="st2", name=f"ps_{name}{g}{nj}")
            for c in range(KC):
                for u in range(2):
                    nc.tensor.matmul(
                        ps[:, u * 512:(u + 1) * 512],
                        w_sb[name][:, c, g, :],
                        xsb[:, c, nj * 1024 + u * 512: nj * 1024 + (u + 1) * 512],
                        start=(c == 0), stop=(c == KC - 1))
            nc.vector.tensor_scalar_add(
                tgt[:, g, nj * 1024:(nj + 1) * 1024], ps, wb_sb[name][:, g:g + 1])

        def proj_half(name, g, nj, u):
            # 512-token half projection in the single-bank "pp" slot, small
            # enough (~1.7us of PE) to slip between attention iterations
            xsb = xsbs[name]
            tgt = {"q": qt, "k": kt, "v": vt}[name]
            lo = nj * 1024 + u * 512
            ps = pp_p.tile([128, 512], F32, tag="pp", name=f"ph_{name}{g}{nj}{u}")
            for c in range(KC):
                nc.tensor.matmul(
                    ps, w_sb[name][:, c, g, :], xsb[:, c, lo:lo + 512],
                    start=(c == 0), stop=(c == KC - 1))
            nc.vector.tensor_scalar_add(
                tgt[:, g, lo:lo + 512], ps, wb_sb[name][:, g:g + 1])

        def transpose_v(g, i):
            # [128 feats, 128 tokens] -> [128 tokens, 2, 64 feats] in vaug
            trp = pp_p.tile([128, 128], BF16, tag="pp", name=f"trp{g}_{i}")
            nc.tensor.transpose(trp, vt[:, g, i * 128:(i + 1) * 128], ident)
            nc.vector.tensor_copy(
                vaug[:, g, :, i, 0:DK],
                trp[:, :].rearrange("p (h f) -> p h f", h=2))

        # startup: everything attn(0, 0..1) needs
        proj("k", 0, 0)
        proj("v", 0, 0)
        for i in range(8):
            transpose_v(0, i)
        proj("q", 0, 0)

        # remaining projection work as a queue of small PE units, popped
        # between attention iterations so ACT never starves. Order matters
        # twice over: a popped unit whose x data hasn't landed blocks the
        # whole PE FIFO, so group (1,0) (token-half 0, resident early) goes
        # first; and group (0,1) is needed by attn(0,2) — Q at its first
        # scores, K/V/transposes at its ki=8.
        def ph(n_, g_, j_, u_):
            return lambda: proj_half(n_, g_, j_, u_)

        def tv(g_, i_):
            return lambda: transpose_v(g_, i_)

        units = []
        for n in ("k", "v"):
            units += [ph(n, 1, 0, u) for u in range(2)]
        units += [tv(1, i) for i in range(8)]
        units += [ph("q", 1, 0, u) for u in range(2)]          # (1,0): 0-13
        units += [ph("q", 0, 1, u) for u in range(2)]          # idx 14-15
        for n in ("k", "v"):
            units += [ph(n, 0, 1, u) for u in range(2)]        # idx 16-19
        units += [tv(0, i) for i in range(8, 16)]              # idx 20-27
        for n in ("k", "v"):
            units += [ph(n, 1, 1, u) for u in range(2)]
        units += [tv(1, i) for i in range(8, 16)]
        units += [ph("q", 1, 1, u) for u in range(2)]          # (1,1): 28-41
        upos = [0]
        TOTAL_SLOTS = sum(4 * qj + 4 for qj in range(NQJ))
        slot_no = [0]

        def pop_units(force_until=None):
            if force_until is not None:
                while upos[0] < force_until:
                    units[upos[0]]()
                    upos[0] += 1
                return
            slot_no[0] += 1
            target = min(len(units),
                         (slot_no[0] * len(units) + TOTAL_SLOTS - 1) // TOTAL_SLOTS)
            while upos[0] < target:
                units[upos[0]]()
                upos[0] += 1

        # ---------------- phase 2: attention ----------------
        def attn(g, qj, pop=False, force=()):
            force = dict(force)
            n_k = n_keytiles(qj)
            ot = [otps.tile([DK + 1, 512], F32, tag="ot", name=f"ot{g}{qj}{h}")
                  for h in range(2)]
            for ki in range(n_k):
                if ki in force:
                    pop_units(force_until=force[ki])
                off = 128 * (ki - 4 * qj) if (mode == "causal" and ki >= 4 * qj) else 0
                st = st2.tile([128, 1024], F32, tag="st2", name=f"st{g}{qj}{ki}")
                for h in range(2):
                    nc.tensor.matmul(
                        st[:, h * 512 + off:(h + 1) * 512],
                        kt[h * DK:(h + 1) * DK, g, ki * 128:(ki + 1) * 128],
                        qt[h * DK:(h + 1) * DK, g, qj * 512 + off: (qj + 1) * 512],
                        start=True, stop=True,
                        tile_position=(h * DK, 0))
                if mode == "general":
                    mt = mk_p.tile([128, 512], F32, tag="mk", name=f"mt{g}{qj}{ki}")
                    nc.sync.dma_start(
                        out=mt,
                        in_=madd[ki * 128:(ki + 1) * 128,
                                 qj * 512:(qj + 1) * 512])
                    for h in range(2):
                        nc.vector.tensor_add(
                            st[:, h * 512:(h + 1) * 512],
                            st[:, h * 512:(h + 1) * 512], mt)
                pt = ptp.tile([128, 1024], BF16, tag="pt", name=f"pt{g}{qj}{ki}")
                if off == 0:
                    nc.scalar.activation(
                        pt, st, mybir.ActivationFunctionType.Exp, scale=SCALE)
                else:
                    # one strided activation covering both heads' live cols
                    nc.scalar.activation(
                        pt[:, :].rearrange("p (h q) -> p h q", h=2)[:, :, off:512],
                        st[:, :].rearrange("p (h q) -> p h q", h=2)[:, :, off:512],
                        mybir.ActivationFunctionType.Exp, scale=SCALE)
                if mode == "causal" and ki >= 4 * qj:
                    for h in range(2):
                        lo = h * 512 + off
                        nc.vector.tensor_mul(
                            pt[:, lo:lo + 128], pt[:, lo:lo + 128], tri_sb)
                if dbg and g == 0 and qj == 1 and ki == 2:
                    stg = out_p.tile([128, 1024], F32, tag="dbgst", name="dbgst",
                                     bufs=1)
                    nc.vector.tensor_copy(stg, st)
                    nc.sync.dma_start(out=dbg_t["dst"][:, :], in_=stg)
                    nc.sync.dma_start(out=dbg_t["dpt"][:, :], in_=pt)
                if pop:
                    pop_units()
                for h in range(2):
                    nc.tensor.matmul(
                        ot[h][:, off:512],
                        vaug[:, g, h, ki, :],
                        pt[:, h * 512 + off:(h + 1) * 512],
                        start=(ki == 0), stop=(ki == n_k - 1),
                        skip_group_check=True)
            # normalize: otn[h*64:(h+1)*64, g, qj, :] = ot[h][0:64] / sums.
            # Per head: copy the sum row (psum row 64) to sbuf, broadcast it
            # over 64 partitions via a K=1 f32r matmul (base 0, no col
            # tiling), approx-reciprocal at base 0, multiply into the merged
            # otn tile (only the TT *output* is partition-shifted, which the
            # plain ops handle; the custom recip op needs base-0 operands).
            if dbg and g == 0 and qj == 1:
                stg2 = out_p.tile([DK + 1, 1024], F32, tag="dbgot", name="dbgot",
                                  bufs=1)
                for h in range(2):
                    nc.vector.tensor_copy(stg2[:, h * 512:(h + 1) * 512], ot[h])
                nc.sync.dma_start(out=dbg_t["dot"][:, :], in_=stg2)
            for h in range(2):
                srow = rcp.tile([1, 512], F32R, tag=f"srow{h}",
                                name=f"srow{g}{qj}{h}")
                nc.vector.tensor_copy(srow, ot[h][DK:DK + 1, :])
                rbc_ps = po.tile([DK, 512], F32, tag="po",
                                 name=f"rbc{g}{qj}{h}")
                nc.tensor.matmul(rbc_ps, onesr_sb[0:1, 0:DK], srow,
                                 start=True, stop=True)
                rbc = rcp.tile([DK, 512], F32, tag=f"rbc{h}",
                               name=f"rbcs{g}{qj}{h}")
                nc.vector.reciprocal_approx_fast(out=rbc, in_=rbc_ps)
                if dbg and g == 0 and qj == 1:
                    nc.sync.dma_start(
                        out=dbg_t["drbc"][h * DK:(h + 1) * DK, :], in_=rbc)
                nc.vector.tensor_mul(
                    otn[h * DK:(h + 1) * DK, g, qj, :],
                    ot[h][0:DK, :], rbc)

        # attention on g0, with the remaining projection work drip-fed into
        # the PE stream between iterations so ACT never starves.
        attn(0, 0, pop=True)
        attn(0, 1, pop=True)
        attn(0, 2, pop=True, force={0: 16, 8: 28})
        attn(0, 3, pop=True)
        pop_units(force_until=len(units))

        # ---------------- phase 3: attention g1 + output projection ----------------
        def oproj(qj):
            # alternate between the po and pp psum slots (pp is free once
            # the projection unit queue has drained) so matmuls on one tile
            # overlap the psum->sbuf copy of the other
            for qb in range(4):
                ob = out_p.tile([128, 1024], BF16, tag="ob", name=f"ob{qj}{qb}")
                for n in range(2):
                    pool_n = po if n == 0 else pp_p
                    pp = pool_n.tile([128, 512], F32,
                                     tag="po" if n == 0 else "pp",
                                     name=f"pp{qj}{qb}{n}")
                    for g in range(NG):
                        nc.tensor.matmul(
                            pp,
                            otn[:, g, qj, qb * 128:(qb + 1) * 128],
                            wo_sb[:, g, n * 512:(n + 1) * 512],
                            start=(g == 0), stop=(g == NG - 1))
                    nc.vector.tensor_copy(ob[:, n * 512:(n + 1) * 512], pp)
                nc.sync.dma_start(
                    out=out[qj * 512 + qb * 128: qj * 512 + (qb + 1) * 128, :],
                    in_=ob)

        # longest q-chunk first so the kernel tail is the shortest one
        for qj in reversed(range(NQJ)):
            attn(1, qj)
            oproj(qj)

        if dbg:
            nc.sync.dma_start(out=dbg_t["dq"][:, :], in_=qt[:, :, :])
            nc.sync.dma_start(out=dbg_t["dk"][:, :], in_=kt[:, :, :])
            nc.sync.dma_start(out=dbg_t["dv"][:, :], in_=vt[:, :, :])
            nc.sync.dma_start(out=dbg_t["dvaug"][:, :], in_=vaug[:, :, :, :, :])
            nc.sync.dma_start(out=dbg_t["dotn"][:, :], in_=otn[:, :, :, :])

    nc.compile()
    return nc


def detect_mode(mask):
    m = np.asarray(mask)[0, 0]
    if (m == np.tril(np.ones((S, S), m.dtype))).all():
        return "causal"
    if (m == 1).all():
        return "ones"
    return "general"


def make_core_inputs(query, key, value, mask, Wq, bq, Wk, bk, Wv, bv, Wo, bo,
                     mode="causal"):
    """Host-side sharding: returns list of per-core input dicts.

    Core c = b * NQUAD + Q owns batch b, heads [4Q, 4Q+4).
    """
    pdt = ml_dtypes.bfloat16
    tri = np.ascontiguousarray(np.triu(np.ones((128, 128), np.float32))).astype(pdt)
    idn = np.ascontiguousarray(np.eye(128, dtype=np.float32)).astype(pdt)
    onesm = np.ones((128, 64), pdt)

    xs = {}
    for b in range(B):
        xs[b] = {
            "xq": np.ascontiguousarray(np.asarray(query)[b].T.astype(pdt)),
            "xk": np.ascontiguousarray(np.asarray(key)[b].T.astype(pdt)),
            "xv": np.ascontiguousarray(np.asarray(value)[b].T.astype(pdt)),
        }
    madd_np = None
    if mode == "general":
        madd_np = np.ascontiguousarray(
            np.where(np.asarray(mask)[0, 0].T == 0, np.float32(-1e30),
                     np.float32(0.0)).astype(np.float32))

    in_maps = []
    for c in range(NCORES):
        b, Q = divmod(c, NQUAD)
        fsl = slice(Q * HPQ * DK, (Q + 1) * HPQ * DK)   # 256 feats of the quad
        m = dict(xs[b])
        m.update({
            "wq": np.ascontiguousarray(
                np.asarray(Wq)[fsl, :].T.astype(pdt).reshape(D, NG, DG)),
            "wk": np.ascontiguousarray(
                np.asarray(Wk)[fsl, :].T.astype(pdt).reshape(D, NG, DG)),
            "wv": np.ascontiguousarray(
                np.asarray(Wv)[fsl, :].T.astype(pdt).reshape(D, NG, DG)),
            "wqb": np.ascontiguousarray(
                np.asarray(bq)[fsl].astype(np.float32).reshape(NG, DG).T),
            "wkb": np.ascontiguousarray(
                np.asarray(bk)[fsl].astype(np.float32).reshape(NG, DG).T),
            "wvb": np.ascontiguousarray(
                np.asarray(bv)[fsl].astype(np.float32).reshape(NG, DG).T),
            "wo": np.ascontiguousarray(
                np.asarray(Wo)[:, fsl].T.astype(pdt).reshape(NG, DG, D)),
            "tri": tri,
            "idn": idn,
            "onesm": onesm,
            "onesr": np.ones((1, 64), np.float32),
        })
        if mode == "general":
            m["madd"] = madd_np
        in_maps.append(m)
    return in_maps


_NC_CACHE = {}


def kernel(query, key, value, mask, Wq, bq, Wk, bk, Wv, bv, Wo, bo,
           trace=False):
    from concourse.bass_utils import run_bass_kernel_spmd

    mode = detect_mode(mask)
    if mode not in _NC_CACHE:
        _NC_CACHE[mode] = build_kernel(mode=mode)
    nc = _NC_CACHE[mode]
    in_maps = make_core_inputs(
        query, key, value, mask, Wq, bq, Wk, bk, Wv, bv, Wo, bo, mode=mode)
    res = run_bass_kernel_spmd(nc, in_maps, core_ids=list(range(NCORES)),
                               trace=trace)
    out = np.zeros((B, S, D), np.float32)
    for c, r in enumerate(res.results):
        b = c // NQUAD
        out[b] += r["out"].astype(np.float32)
    out += np.asarray(bo).astype(np.float32)[None, None, :]
    if trace:
        kernel.last_results = res
    return out


# revision 51
# speedup vs baseline: 1.1229x; 1.0252x over previous
"""
Multi-head attention (B=2, S=2048, D=1024, H=16, causal) on 8 Trainium2
NeuronCores via Bass/Tile.

Sharding: batch x head-quad. Core (b, Q) owns batch b and heads
[4Q, 4Q+4) (two groups of 2 heads), so each core reads only its batch's
q/k/v inputs (12 MB bf16 vs 24 MB for pure head sharding) and writes a
[2048, 1024] bf16 partial output. The host sums the 4 quad-partials per
batch and adds bo (the "all-reduce after w_o" done host-side since the
kernel contract is full-in / full-out).

All matmuls run in bf16 (2x the fp32r rate, FWL weight loads):
  Q^T/K^T/V^T   [128 feats (2 heads x 64), g, 2048 tokens]
  scores        S^T tiles [128 keys, q] via row-packed per-head matmuls
  softmax       exp on ACT (scale=1/8 folded in, no max-subtraction:
                |s/8| <~ 6), row sums via a ones column appended to V
                (psum row 64), reciprocal_approx_fast on DVE, partition
                broadcast via a K=2 selector matmul
  P@V           vaug [128 tokens, 65] stationary per head; diagonal
                tiles stream only the live [off:512] columns
  out proj      otn [128 feats, 512 q] merged across heads -> K=128
                matmuls against wo [128, 1024], accumulated over groups
"""

import os
import sys

for _p in ("/opt/trn_rl_repo", "/root/.axon_site/_ro/trn_rl_repo"):
    if os.path.isdir(_p) and _p not in sys.path:
        sys.path.insert(0, _p)

import numpy as np
import ml_dtypes
from contextlib import ExitStack

import concourse.bass as bass
import concourse.tile as tile
from concourse import bacc
from concourse import mybir

B, S, D, H = 2, 2048, 1024, 16
DK = D // H                # 64
NCORES = 8
NQUAD = 4                  # head quads
HPQ = H // NQUAD           # 4 heads per core
NG = 2                     # feature groups per core (2 heads each)
DG = 2 * DK                # 128 feats per group
SCALE = 1.0 / np.sqrt(DK)  # 0.125

KC = D // 128              # 8 contraction chunks for projections
NJP = S // 1024            # 2 token chunks of 1024
NQJ = S // 512             # 4 q chunks of 512
NKT = S // 128             # 16 k tiles of 128

F32 = mybir.dt.float32
F32R = mybir.dt.float32r
BF16 = mybir.dt.bfloat16


def build_kernel(mode="causal", dbg=False):
    """Identical program on all cores; per-core slices arrive as data.

    mode: "causal" (skip upper-triangular key tiles, tri-mask diagonal),
          "ones" (no masking), "general" (additive mask streamed from DRAM).
    """
    nc = bacc.Bacc()

    xq = nc.declare_dram_parameter("xq", [D, S], BF16, isOutput=False)
    xk = nc.declare_dram_parameter("xk", [D, S], BF16, isOutput=False)
    xv = nc.declare_dram_parameter("xv", [D, S], BF16, isOutput=False)
    wq = nc.declare_dram_parameter("wq", [D, NG, DG], BF16, isOutput=False)
    wk = nc.declare_dram_parameter("wk", [D, NG, DG], BF16, isOutput=False)
    wv = nc.declare_dram_parameter("wv", [D, NG, DG], BF16, isOutput=False)
    wqb = nc.declare_dram_parameter("wqb", [DG, NG], F32, isOutput=False)
    wkb = nc.declare_dram_parameter("wkb", [DG, NG], F32, isOutput=False)
    wvb = nc.declare_dram_parameter("wvb", [DG, NG], F32, isOutput=False)
    wo = nc.declare_dram_parameter("wo", [NG, DG, D], BF16, isOutput=False)
    tri = nc.declare_dram_parameter("tri", [128, 128], BF16, isOutput=False)
    idn = nc.declare_dram_parameter("idn", [128, 128], BF16, isOutput=False)
    onesm = nc.declare_dram_parameter("onesm", [128, 64], BF16, isOutput=False)
    onesr = nc.declare_dram_parameter("onesr", [1, 64], F32R, isOutput=False)
    madd = None
    if mode == "general":
        madd = nc.declare_dram_parameter("madd", [S, S], F32, isOutput=False)
    out = nc.declare_dram_parameter("out", [S, D], BF16, isOutput=True)
    dbg_t = {}
    if dbg:
        for dn, shape, dt in (
                ("dq", [128, NG * S], BF16), ("dk", [128, NG * S], BF16),
                ("dv", [128, NG * S], BF16),
                ("dvaug", [128, NG * 2 * NKT * (DK + 1)], BF16),
                ("dst", [128, 1024], F32), ("dpt", [128, 1024], BF16),
                ("dot", [DK + 1, 1024], F32),
                ("drbc", [128, 512], F32),
                ("dotn", [128, NG * NQJ * 512], BF16)):
            dbg_t[dn] = nc.declare_dram_parameter(dn, shape, dt, isOutput=True)

    def n_keytiles(qj):
        return 4 * qj + 4 if mode == "causal" else NKT

    with tile.TileContext(nc) as tc, ExitStack() as ctx:
        persist = ctx.enter_context(tc.tile_pool(name="persist", bufs=1))
        ptp = ctx.enter_context(tc.tile_pool(name="ptp", bufs=3))
        rcp = ctx.enter_context(tc.tile_pool(name="rcp", bufs=2))
        out_p = ctx.enter_context(tc.tile_pool(name="outp", bufs=4))
        mk_p = None
        if mode == "general":
            mk_p = ctx.enter_context(tc.tile_pool(name="mk", bufs=4))
        # PSUM: st2 2 banks x 2 bufs + ot 1 bank x 2 + po 1 + pp 1 = 8 banks
        st2 = ctx.enter_context(
            tc.tile_pool(name="st2", bufs=2, space=bass.MemorySpace.PSUM))
        otps = ctx.enter_context(
            tc.tile_pool(name="otps", bufs=2, space=bass.MemorySpace.PSUM))
        po = ctx.enter_context(
            tc.tile_pool(name="po", bufs=1, space=bass.MemorySpace.PSUM))
        pp_p = ctx.enter_context(
            tc.tile_pool(name="ppp", bufs=1, space=bass.MemorySpace.PSUM))

        # ---------------- persistent tiles ----------------
        qt = persist.tile([128, NG, S], BF16)       # Q^T
        kt = persist.tile([128, NG, S], BF16)       # K^T
        vt = persist.tile([128, NG, S], BF16)       # V^T (consumed by transpose)
        # V augmented: [128 tokens, g, head, ktile, 65]; col 64 == 1.0
        vaug = persist.tile([128, NG, 2, NKT, DK + 1], BF16)
        # normalized attention outputs, [128 feats(2 heads), g, qj, 512 q]
        otn = persist.tile([128, NG, NQJ, 512], BF16)
        wo_sb = persist.tile([128, NG, D], BF16)
        tri_sb = persist.tile([128, 128], BF16)
        ident = persist.tile([128, 128], BF16)
        ones_sb = persist.tile([128, 64], BF16)
        onesr_sb = persist.tile([1, 64], F32R)
        xq_sb = persist.tile([128, KC, S], BF16)
        xk_sb = persist.tile([128, KC, S], BF16)
        xv_sb = persist.tile([128, KC, S], BF16)

        # DMA emission order = consumption order. Each DMA_DIRECT2D costs
        # ~0.6us of Sync-engine issue time, so the first projection's
        # operands (wk + xk token-half 0) go first; constants and token-half
        # 1 (as one large transfer per tensor) follow.
        w_sb = {}
        wb_sb = {}
        xsbs = {"q": xq_sb, "k": xk_sb, "v": xv_sb}
        xsrcs = {"q": xq, "k": xk, "v": xv}

        def load_w(name, wsrc, wbsrc):
            wt = persist.tile([128, KC, NG, DG], BF16, name=f"w{name}")
            nc.sync.dma_start(
                out=wt, in_=wsrc[:, :, :].rearrange("(c p) g n -> p c g n",
                                                    p=128))
            bt = persist.tile([DG, NG], F32, name=f"wb{name}")
            nc.sync.dma_start(out=bt, in_=wbsrc[:, :])
            w_sb[name] = wt
            wb_sb[name] = bt

        def load_x_chunks(name, nj):
            # alternate the HWDGE issuing engine (Sync / Scalar) — each
            # issue costs ~0.8us of engine time and ACT is idle at startup,
            # so splitting halves the issue-serialization latency
            for c in range(KC):
                eng = nc.sync if c % 2 == 0 else nc.scalar
                eng.dma_start(
                    out=xsbs[name][:, c, nj * 1024:(nj + 1) * 1024],
                    in_=xsrcs[name][c * 128:(c + 1) * 128,
                                    nj * 1024:(nj + 1) * 1024])

        load_w("k", wk, wkb)
        load_x_chunks("k", 0)
        load_w("v", wv, wvb)
        load_x_chunks("v", 0)
        nc.sync.dma_start(out=ident, in_=idn[:, :])
        load_w("q", wq, wqb)
        load_x_chunks("q", 0)
        nc.sync.dma_start(out=ones_sb, in_=onesm[:, :])
        nc.sync.dma_start(out=onesr_sb, in_=onesr[:, :])
        nc.sync.dma_start(out=tri_sb, in_=tri[:, :])
        # token-half 1: one 2 MB transfer per tensor
        for name in ("k", "v", "q"):
            nc.sync.dma_start(
                out=xsbs[name][:, :, 1024:2048],
                in_=xsrcs[name][:, 1024:2048].rearrange("(c p) t -> p c t",
                                                        p=128))
        nc.sync.dma_start(out=wo_sb, in_=wo[:, :, :].rearrange("g p n -> p g n"))

        # ones column of vaug
        nc.vector.tensor_copy(
            vaug[:, :, :, :, DK:DK + 1], ones_sb[:, 0:NG * 2 * NKT])

        # ---------------- phase 1: QKV projections ----------------
        def proj(name, g, nj):
            # full [128, 1024] projection via the double-buffered st2 slots
            # (used only for the startup unit, before attention exists)
            xsb = xsbs[name]
            tgt = {"q": qt, "k": kt, "v": vt}[name]
            ps = st2.tile([128, 1024], F32, tag="st2", name=f"ps_{name}{g}{nj}")
            for c in range(KC):
                for u in range(2):
                    nc.tensor.matmul(
                        ps[:, u * 512:(u + 1) * 512],
                        w_sb[name][:, c, g, :],
                        xsb[:, c, nj * 1024 + u * 512: nj * 1024 + (u + 1) * 512],
                        start=(c == 0), stop=(c == KC - 1))
            nc.vector.tensor_scalar_add(
                tgt[:, g, nj * 1024:(nj + 1) * 1024], ps, wb_sb[name][:, g:g + 1])

        def proj_half(name, g, nj, u):
            # 512-token half projection in the single-bank "pp" slot, small
            # enough (~1.7us of PE) to slip between attention iterations
            xsb = xsbs[name]
            tgt = {"q": qt, "k": kt, "v": vt}[name]
            lo = nj * 1024 + u * 512
            ps = pp_p.tile([128, 512], F32, tag="pp", name=f"ph_{name}{g}{nj}{u}")
            for c in range(KC):
                nc.tensor.matmul(
                    ps, w_sb[name][:, c, g, :], xsb[:, c, lo:lo + 512],
                    start=(c == 0), stop=(c == KC - 1))
            nc.vector.tensor_scalar_add(
                tgt[:, g, lo:lo + 512], ps, wb_sb[name][:, g:g + 1])

        def transpose_v(g, i):
            # [128 feats, 128 tokens] -> [128 tokens, 2, 64 feats] in vaug
            trp = pp_p.tile([128, 128], BF16, tag="pp", name=f"trp{g}_{i}")
            nc.tensor.transpose(trp, vt[:, g, i * 128:(i + 1) * 128], ident)
            nc.vector.tensor_copy(
                vaug[:, g, :, i, 0:DK],
                trp[:, :].rearrange("p (h f) -> p h f", h=2))

        # startup: everything attn(0, 0..1) needs
        proj("k", 0, 0)
        proj("v", 0, 0)
        for i in range(8):
            transpose_v(0, i)
        proj("q", 0, 0)

        # remaining projection work as a queue of small PE units, popped
        # between attention iterations so ACT never starves. Order matters
        # twice over: a popped unit whose x data hasn't landed blocks the
        # whole PE FIFO, so group (1,0) (token-half 0, resident early) goes
        # first; and group (0,1) is needed by attn(0,2) — Q at its first
        # scores, K/V/transposes at its ki=8.
        def ph(n_, g_, j_, u_):
            return lambda: proj_half(n_, g_, j_, u_)

        def tv(g_, i_):
            return lambda: transpose_v(g_, i_)

        # phase-2 queue holds groups (1,0) and (0,1); group (1,1) defers to
        # a phase-3 queue consumed just-in-time during attn(1, 0..2), which
        # moves ~13us of PE work out of the PE-bound phase 2 into phase 3's
        # ACT-covered slack.
        units = []
        for n in ("k", "v"):
            units += [ph(n, 1, 0, u) for u in range(2)]
        units += [tv(1, i) for i in range(8)]
        units += [ph("q", 1, 0, u) for u in range(2)]          # (1,0): 0-13
        units += [ph("q", 0, 1, u) for u in range(2)]          # idx 14-15
        for n in ("k", "v"):
            units += [ph(n, 0, 1, u) for u in range(2)]        # idx 16-19
        units += [tv(0, i) for i in range(8, 16)]              # idx 20-27
        units3 = [ph("q", 1, 1, 0)]                            # idx 0
        units3 += [ph(n, 1, 1, 0) for n in ("k", "v")]         # idx 1-2
        units3 += [tv(1, i) for i in range(8, 12)]             # idx 3-6
        units3 += [ph("q", 1, 1, 1)]                           # idx 7
        units3 += [ph(n, 1, 1, 1) for n in ("k", "v")]         # idx 8-9
        units3 += [tv(1, i) for i in range(12, 16)]            # idx 10-13
        state = {"units": units, "upos": 0,
                 "slots": sum(4 * qj + 4 for qj in range(NQJ)), "slot": 0}

        def pop_units(force_until=None):
            if force_until is not None:
                while state["upos"] < force_until:
                    state["units"][state["upos"]]()
                    state["upos"] += 1
                return
            state["slot"] += 1
            target = min(len(state["units"]),
                         (state["slot"] * len(state["units"]) + state["slots"] - 1)
                         // state["slots"])
            while state["upos"] < target:
                state["units"][state["upos"]]()
                state["upos"] += 1

        def switch_queue(new_units, slots):
            assert state["upos"] == len(state["units"])
            state.update(units=new_units, upos=0, slots=slots, slot=0)

        # ---------------- phase 2: attention ----------------
        def attn(g, qj, pop=False, force=()):
            force = dict(force)
            n_k = n_keytiles(qj)
            ot = [otps.tile([DK + 1, 512], F32, tag="ot", name=f"ot{g}{qj}{h}")
                  for h in range(2)]
            for ki in range(n_k):
                if ki in force:
                    pop_units(force_until=force[ki])
                off = 128 * (ki - 4 * qj) if (mode == "causal" and ki >= 4 * qj) else 0
                st = st2.tile([128, 1024], F32, tag="st2", name=f"st{g}{qj}{ki}")
                for h in range(2):
                    nc.tensor.matmul(
                        st[:, h * 512 + off:(h + 1) * 512],
                        kt[h * DK:(h + 1) * DK, g, ki * 128:(ki + 1) * 128],
                        qt[h * DK:(h + 1) * DK, g, qj * 512 + off: (qj + 1) * 512],
                        start=True, stop=True,
                        tile_position=(h * DK, 0))
                if mode == "general":
                    mt = mk_p.tile([128, 512], F32, tag="mk", name=f"mt{g}{qj}{ki}")
                    nc.sync.dma_start(
                        out=mt,
                        in_=madd[ki * 128:(ki + 1) * 128,
                                 qj * 512:(qj + 1) * 512])
                    for h in range(2):
                        nc.vector.tensor_add(
                            st[:, h * 512:(h + 1) * 512],
                            st[:, h * 512:(h + 1) * 512], mt)
                pt = ptp.tile([128, 1024], BF16, tag="pt", name=f"pt{g}{qj}{ki}")
                if off == 0:
                    nc.scalar.activation(
                        pt, st, mybir.ActivationFunctionType.Exp, scale=SCALE)
                else:
                    # one strided activation covering both heads' live cols
                    nc.scalar.activation(
                        pt[:, :].rearrange("p (h q) -> p h q", h=2)[:, :, off:512],
                        st[:, :].rearrange("p (h q) -> p h q", h=2)[:, :, off:512],
                        mybir.ActivationFunctionType.Exp, scale=SCALE)
                if mode == "causal" and ki >= 4 * qj:
                    for h in range(2):
                        lo = h * 512 + off
                        nc.vector.tensor_mul(
                            pt[:, lo:lo + 128], pt[:, lo:lo + 128], tri_sb)
                if dbg and g == 0 and qj == 1 and ki == 2:
                    stg = out_p.tile([128, 1024], F32, tag="dbgst", name="dbgst",
                                     bufs=1)
                    nc.vector.tensor_copy(stg, st)
                    nc.sync.dma_start(out=dbg_t["dst"][:, :], in_=stg)
                    nc.sync.dma_start(out=dbg_t["dpt"][:, :], in_=pt)
                if pop:
                    pop_units()
                for h in range(2):
                    nc.tensor.matmul(
                        ot[h][:, off:512],
                        vaug[:, g, h, ki, :],
                        pt[:, h * 512 + off:(h + 1) * 512],
                        start=(ki == 0), stop=(ki == n_k - 1),
                        skip_group_check=True)
            # normalize: otn[h*64:(h+1)*64, g, qj, :] = ot[h][0:64] / sums.
            # Per head: copy the sum row (psum row 64) to sbuf, broadcast it
            # over 64 partitions via a K=1 f32r matmul (base 0, no col
            # tiling), approx-reciprocal at base 0, multiply into the merged
            # otn tile (only the TT *output* is partition-shifted, which the
            # plain ops handle; the custom recip op needs base-0 operands).
            if dbg and g == 0 and qj == 1:
                stg2 = out_p.tile([DK + 1, 1024], F32, tag="dbgot", name="dbgot",
                                  bufs=1)
                for h in range(2):
                    nc.vector.tensor_copy(stg2[:, h * 512:(h + 1) * 512], ot[h])
                nc.sync.dma_start(out=dbg_t["dot"][:, :], in_=stg2)
            for h in range(2):
                srow = rcp.tile([1, 512], F32R, tag=f"srow{h}",
                                name=f"srow{g}{qj}{h}")
                nc.vector.tensor_copy(srow, ot[h][DK:DK + 1, :])
                rbc_ps = po.tile([DK, 512], F32, tag="po",
                                 name=f"rbc{g}{qj}{h}")
                nc.tensor.matmul(rbc_ps, onesr_sb[0:1, 0:DK], srow,
                                 start=True, stop=True)
                rbc = rcp.tile([DK, 512], F32, tag=f"rbc{h}",
                               name=f"rbcs{g}{qj}{h}")
                nc.vector.reciprocal_approx_fast(out=rbc, in_=rbc_ps)
                if dbg and g == 0 and qj == 1:
                    nc.sync.dma_start(
                        out=dbg_t["drbc"][h * DK:(h + 1) * DK, :], in_=rbc)
                nc.vector.tensor_mul(
                    otn[h * DK:(h + 1) * DK, g, qj, :],
                    ot[h][0:DK, :], rbc)

        # attention on g0, with the remaining projection work drip-fed into
        # the PE stream between iterations so ACT never starves.
        attn(0, 0, pop=True)
        attn(0, 1, pop=True)
        attn(0, 2, pop=True, force={0: 16, 8: 28})
        attn(0, 3, pop=True)
        pop_units(force_until=len(units))

        # ---------------- phase 3: attention g1 + output projection ----------------
        def oproj(qj):
            # alternate between the po and pp psum slots (pp is free once
            # the projection unit queue has drained) so matmuls on one tile
            # overlap the psum->sbuf copy of the other
            for qb in range(4):
                ob = out_p.tile([128, 1024], BF16, tag="ob", name=f"ob{qj}{qb}")
                for n in range(2):
                    pool_n = po if n == 0 else pp_p
                    pp = pool_n.tile([128, 512], F32,
                                     tag="po" if n == 0 else "pp",
                                     name=f"pp{qj}{qb}{n}")
                    for g in range(NG):
                        nc.tensor.matmul(
                            pp,
                            otn[:, g, qj, qb * 128:(qb + 1) * 128],
                            wo_sb[:, g, n * 512:(n + 1) * 512],
                            start=(g == 0), stop=(g == NG - 1))
                    nc.vector.tensor_copy(ob[:, n * 512:(n + 1) * 512], pp)
                nc.sync.dma_start(
                    out=out[qj * 512 + qb * 128: qj * 512 + (qb + 1) * 128, :],
                    in_=ob)

        # phase 3: normal order, draining the (1,1) unit queue just-in-time
        switch_queue(units3, sum(4 * qj + 4 for qj in range(NQJ - 1)))
        attn(1, 0, pop=True)
        oproj(0)
        attn(1, 1, pop=True)
        oproj(1)
        attn(1, 2, pop=True, force={0: 1, 8: 7})
        oproj(2)
        pop_units(force_until=len(units3))
        attn(1, 3)
        oproj(3)

        if dbg:
            nc.sync.dma_start(out=dbg_t["dq"][:, :], in_=qt[:, :, :])
            nc.sync.dma_start(out=dbg_t["dk"][:, :], in_=kt[:, :, :])
            nc.sync.dma_start(out=dbg_t["dv"][:, :], in_=vt[:, :, :])
            nc.sync.dma_start(out=dbg_t["dvaug"][:, :], in_=vaug[:, :, :, :, :])
            nc.sync.dma_start(out=dbg_t["dotn"][:, :], in_=otn[:, :, :, :])

    nc.compile()
    return nc


def detect_mode(mask):
    m = np.asarray(mask)[0, 0]
    if (m == np.tril(np.ones((S, S), m.dtype))).all():
        return "causal"
    if (m == 1).all():
        return "ones"
    return "general"


def make_core_inputs(query, key, value, mask, Wq, bq, Wk, bk, Wv, bv, Wo, bo,
                     mode="causal"):
    """Host-side sharding: returns list of per-core input dicts.

    Core c = b * NQUAD + Q owns batch b, heads [4Q, 4Q+4).
    """
    pdt = ml_dtypes.bfloat16
    tri = np.ascontiguousarray(np.triu(np.ones((128, 128), np.float32))).astype(pdt)
    idn = np.ascontiguousarray(np.eye(128, dtype=np.float32)).astype(pdt)
    onesm = np.ones((128, 64), pdt)

    xs = {}
    for b in range(B):
        xs[b] = {
            "xq": np.ascontiguousarray(np.asarray(query)[b].T.astype(pdt)),
            "xk": np.ascontiguousarray(np.asarray(key)[b].T.astype(pdt)),
            "xv": np.ascontiguousarray(np.asarray(value)[b].T.astype(pdt)),
        }
    madd_np = None
    if mode == "general":
        madd_np = np.ascontiguousarray(
            np.where(np.asarray(mask)[0, 0].T == 0, np.float32(-1e30),
                     np.float32(0.0)).astype(np.float32))

    in_maps = []
    for c in range(NCORES):
        b, Q = divmod(c, NQUAD)
        fsl = slice(Q * HPQ * DK, (Q + 1) * HPQ * DK)   # 256 feats of the quad
        m = dict(xs[b])
        m.update({
            "wq": np.ascontiguousarray(
                np.asarray(Wq)[fsl, :].T.astype(pdt).reshape(D, NG, DG)),
            "wk": np.ascontiguousarray(
                np.asarray(Wk)[fsl, :].T.astype(pdt).reshape(D, NG, DG)),
            "wv": np.ascontiguousarray(
                np.asarray(Wv)[fsl, :].T.astype(pdt).reshape(D, NG, DG)),
            "wqb": np.ascontiguousarray(
                np.asarray(bq)[fsl].astype(np.float32).reshape(NG, DG).T),
            "wkb": np.ascontiguousarray(
                np.asarray(bk)[fsl].astype(np.float32).reshape(NG, DG).T),
            "wvb": np.ascontiguousarray(
                np.asarray(bv)[fsl].astype(np.float32).reshape(NG, DG).T),
            "wo": np.ascontiguousarray(
                np.asarray(Wo)[:, fsl].T.astype(pdt).reshape(NG, DG, D)),
            "tri": tri,
            "idn": idn,
            "onesm": onesm,
            "onesr": np.ones((1, 64), np.float32),
        })
        if mode == "general":
            m["madd"] = madd_np
        in_maps.append(m)
    return in_maps


_NC_CACHE = {}


def kernel(query, key, value, mask, Wq, bq, Wk, bk, Wv, bv, Wo, bo,
           trace=False):
    from concourse.bass_utils import run_bass_kernel_spmd

    mode = detect_mode(mask)
    if mode not in _NC_CACHE:
        _NC_CACHE[mode] = build_kernel(mode=mode)
    nc = _NC_CACHE[mode]
    in_maps = make_core_inputs(
        query, key, value, mask, Wq, bq, Wk, bk, Wv, bv, Wo, bo, mode=mode)
    res = run_bass_kernel_spmd(nc, in_maps, core_ids=list(range(NCORES)),
                               trace=trace)
    out = np.zeros((B, S, D), np.float32)
    for c, r in enumerate(res.results):
        b = c // NQUAD
        out[b] += r["out"].astype(np.float32)
    out += np.asarray(bo).astype(np.float32)[None, None, :]
    if trace:
        kernel.last_results = res
    return out
